# revision 15
# baseline (speedup 1.0000x reference)
"""Distributed causal multi-head attention for TRN2 (8 NeuronCores), v2.

Sharding: core c handles batch c//2. The batch's 2048 query rows are cut
into eight 256-row position blocks; even cores own blocks {0,2,4,6}, odd
cores {1,3,5,7}. Chunk slot s (0..3) statically runs 4(s+1) key tiles of
128, which covers both parities (even cores pad, odd cores fit exactly) —
identical instruction streams on all cores, causal skip of ~40% of the
rectangle. The causal mask is applied only on the last 4 key tiles of
each slot via (iota >= mstart) * exp with per-core mstart data.

Matmuls run in bf16 (fp32 PSUM accumulation); softmax stays max-free
(scores are O(1) for randn inputs) with the denominator accumulated by a
ones-column folded into the AV matmul.
"""

import sys

sys.path.insert(0, "/opt/trn_rl_repo")
import numpy as np
import ml_dtypes
import concourse.bass as bass
import concourse.mybir as mybir
import concourse.tile as tile
from concourse.vector_clock import ScopedClock
from concourse.bass_utils import run_bass_kernel_spmd
import concourse.bass_utils as _bu
import os as _os

if not getattr(_bu, "_ldw_patch", False):
    _bu._ldw_patch = True
    _orig_run_command = _bu.run_command

    def _run_command_ldw(cmd, *a, **kw):
        if _os.environ.get("BASS_LDW_OPT") and isinstance(cmd, list):
            cmd = [c.replace("--enable-ldw-opt=false", "--enable-ldw-opt=true") for c in cmd]
        return _orig_run_command(cmd, *a, **kw)

    _bu.run_command = _run_command_ldw

B, N, DIM = 4, 2048, 1024
HEADS, DH = 16, 64
INNER = HEADS * DH
SCALE = DH ** -0.5
NQ = 1024
CH = 256          # query chunk rows
NSLOT = 4         # chunks per core
F32 = mybir.dt.float32
BF16 = mybir.dt.bfloat16
AF = mybir.ActivationFunctionType
ALU = mybir.AluOpType

LAST_RESULT = None


def _drain_and_barrier_patched(self, tick_clock, wait_clock):
    nop_inst = self.nc.sync.nop(nofuse=True)
    wait_clock.add_sem_waits(nop_inst.ins, ScopedClock({None: tick_clock.global_clock}))
    si = nop_inst.ins.sync_info
    waits = list(si.on_wait or []) if si else []
    if len(waits) > 1:
        nop_inst.ins.sync_info = mybir.SyncInfo(
            on_wait=waits[:1], on_update=list(si.on_update or [])
        )
        for i in range(1, len(waits)):
            extra = self.nc.sync.nop(nofuse=True)
            extra.ins.sync_info = mybir.SyncInfo(on_wait=[waits[i]], on_update=[])
    self.nc.sync.drain()
    self.nc.all_engine_barrier()
    popped = self.nc._tile_sem_poison_stack.pop()
    assert popped is self._sem_poison
    self.nc.clear_and_free_semaphores(list(self.sems.allocated().values()))
    self.nc.all_engine_barrier()


tile.TileContext._drain_and_barrier = _drain_and_barrier_patched


def _split_multi_waits(nc):
    for f in nc.m.functions:
        for bb in f.blocks:
            insts = bb.instructions
            if not any(
                i.sync_info and i.sync_info.on_wait and len(i.sync_info.on_wait) > 1
                for i in insts
            ):
                continue
            new = []
            for inst in insts:
                si = inst.sync_info
                waits = list(si.on_wait) if si and si.on_wait else []
                if len(waits) > 1:
                    for w in waits[:-1]:
                        nop = mybir.InstNoOp(
                            name=nc.get_next_instruction_name(), ins=[], outs=[]
                        )
                        nop.engine = inst.engine
                        nop.sync_info = mybir.SyncInfo(on_wait=[w], on_update=[])
                        new.append(nop)
                    inst.sync_info = mybir.SyncInfo(
                        on_wait=[waits[-1]], on_update=list(si.on_update or [])
                    )
                new.append(inst)
            bb.instructions = new


def _act_reciprocal(nc, out, in_):
    # ACT LUT reciprocal (~1e-3 rel; fine for softmax denominators). bass's
    # activation() refuses Reciprocal, so emit the instruction directly.
    se = nc.scalar
    ins = [se.lower_ap(in_)]
    for v in (0.0, 1.0, 0.0):
        ins.append(mybir.ImmediateValue(dtype=mybir.dt.float32, value=v))
    return se.add_instruction(
        mybir.InstActivation(
            name=nc.get_next_instruction_name(),
            func=mybir.ActivationFunctionType.Reciprocal,
            ins=ins,
            outs=[se.lower_ap(out)],
        )
    )


def build_graph():
    nc = bass.Bass("TRN2", target_bir_lowering=False)

    p_xT = nc.declare_dram_parameter("xT", [DIM, N], BF16, isOutput=False)
    p_xTq = nc.declare_dram_parameter("xTq", [DIM, NQ], BF16, isOutput=False)
    p_wq = nc.declare_dram_parameter("w_q", [DIM, INNER], BF16, isOutput=False)
    p_wkv = nc.declare_dram_parameter("w_kv", [DIM, 2 * INNER], BF16, isOutput=False)
    p_wbo = nc.declare_dram_parameter("wb_out", [INNER + 1, DIM], BF16, isOutput=False)
    p_ms = nc.declare_dram_parameter("mstart", [128, NSLOT * 16], F32, isOutput=False)
    p_iota = nc.declare_dram_parameter("iota", [128, CH], F32, isOutput=False)
    p_out = nc.declare_dram_parameter("out", [NQ, DIM], F32, isOutput=True)

    with tile.TileContext(nc) as tc:
        with (
            tc.tile_pool(name="const", bufs=1) as cst,
            tc.tile_pool(name="qt", bufs=1) as qtp,
            tc.tile_pool(name="vsb", bufs=1) as vsp,
            tc.tile_pool(name="ktr", bufs=1) as ktrp,
        ):
            iota = cst.tile([128, CH], F32, tag="iota", name="iota")
            nc.sync.dma_start(iota[:, :], p_iota[:, :])
            ms = cst.tile([128, NSLOT * 16], F32, tag="ms", name="ms")
            nc.sync.dma_start(ms[:, :], p_ms[:, :])
            ones64 = cst.tile([1, 64], F32, tag="ones64", name="ones64")
            nc.vector.memset(ones64[:, :], 1.0)
            onesb = cst.tile([1, 1024], BF16, tag="onesb", name="onesb")
            nc.vector.memset(onesb[:, :], 1.0)

            qt = [qtp.tile([128, NQ], BF16, tag=f"qt{i}", name=f"qt{i}") for i in range(8)]
            vsb = [vsp.tile([128, HEADS * (DH + 1)], BF16, tag=f"v{i}", name=f"v{i}") for i in range(16)]
            ktr = [ktrp.tile([128, N], BF16, tag=f"kt{i}", name=f"kt{i}") for i in range(8)]

            # ---------------- P0a: QT = w_q.T @ xTq ----------------
            with (
                tc.tile_pool(name="xtq", bufs=1) as xtqp,
                tc.tile_pool(name="wqp", bufs=1) as wqp,
                tc.tile_pool(name="ps0", bufs=2, space="PSUM") as ps0,
            ):
                xtq = [xtqp.tile([128, NQ], BF16, tag=f"xtq{i}", name=f"xtq{i}") for i in range(8)]
                for i in range(8):
                    nc.sync.dma_start(xtq[i][:, :], p_xTq[i * 128:(i + 1) * 128, :])
                wq = [wqp.tile([128, INNER], BF16, tag=f"wq{i}", name=f"wq{i}") for i in range(8)]
                for i in range(8):
                    nc.sync.dma_start(wq[i][:, :], p_wq[i * 128:(i + 1) * 128, :])
                for ft in range(8):
                    for tc2 in range(2):
                        pq = ps0.tile([128, 512], F32, name="pq")
                        for kt in range(8):
                            nc.tensor.matmul(
                                pq[:, :],
                                wq[kt][:, ft * 128:(ft + 1) * 128],
                                xtq[kt][:, tc2 * 512:(tc2 + 1) * 512],
                                start=(kt == 0),
                                stop=(kt == 7),
                            )
                        nc.scalar.activation(
                            qt[ft][:, tc2 * 512:(tc2 + 1) * 512], pq[:, :], AF.Copy
                        )

            # ---------------- P0b/P0c: KT (resident) and V ----------------
            with tc.tile_pool(name="xt", bufs=1) as xtp:
                xt = [xtp.tile([128, N], BF16, tag=f"xt{i}", name=f"xt{i}") for i in range(8)]
                for i in range(8):
                    nc.sync.dma_start(xt[i][:, :], p_xT[i * 128:(i + 1) * 128, :])

                with (
                    tc.tile_pool(name="wkp", bufs=3) as wkp,
                    tc.tile_pool(name="ps1", bufs=1, space="PSUM") as ps1,
                ):
                    for ft in range(8):
                        pk = [ps1.tile([128, 512], F32, tag=f"pk{j}", name=f"pk{j}") for j in range(4)]
                        for kt in range(8):
                            wk = wkp.tile([128, 128], BF16, tag="wk", name="wk")
                            nc.sync.dma_start(
                                wk[:, :],
                                p_wkv[kt * 128:(kt + 1) * 128, ft * 128:(ft + 1) * 128],
                            )
                            for tc4 in range(4):
                                nc.tensor.matmul(
                                    pk[tc4][:, :],
                                    wk[:, :],
                                    xt[kt][:, tc4 * 512:(tc4 + 1) * 512],
                                    start=(kt == 0),
                                    stop=(kt == 7),
                                )
                        for tc4 in range(4):
                            nc.scalar.activation(
                                ktr[ft][:, tc4 * 512:(tc4 + 1) * 512], pk[tc4][:, :], AF.Copy
                            )

                with (
                    tc.tile_pool(name="wvp", bufs=3) as wvp,
                    tc.tile_pool(name="ps2", bufs=1, space="PSUM") as ps2,
                ):
                    for tgrp in range(2):
                        for fc in range(2):
                            pv = [ps2.tile([128, 512], F32, tag=f"pv{j}", name=f"pv{j}") for j in range(8)]
                            for kt in range(8):
                                wv = wvp.tile([128, 512], BF16, tag="wv", name="wv")
                                nc.sync.dma_start(
                                    wv[:, :],
                                    p_wkv[
                                        kt * 128:(kt + 1) * 128,
                                        INNER + fc * 512:INNER + (fc + 1) * 512,
                                    ],
                                )
                                for t8 in range(8):
                                    tt = tgrp * 8 + t8
                                    nc.tensor.matmul(
                                        pv[t8][:, :],
                                        xt[kt][:, tt * 128:(tt + 1) * 128],
                                        wv[:, :],
                                        start=(kt == 0),
                                        stop=(kt == 7),
                                    )
                            for t8 in range(8):
                                tt = tgrp * 8 + t8
                                dst = vsb[tt][
                                    :, fc * 8 * 65:(fc * 8 + 8) * 65
                                ].rearrange("p (g d) -> p g d", g=8)[:, :, 0:64]
                                src = pv[t8][:, :].rearrange("p (g d) -> p g d", g=8)
                                nc.vector.tensor_copy(dst, src)
                    for tt in range(16):
                        nc.vector.memset(
                            vsb[tt][:, :].rearrange("p (g d) -> p g d", g=16)[:, :, 64:65],
                            1.0,
                        )

            # ---------------- P1: attention ----------------
            afp = tc.alloc_tile_pool(name="af", bufs=1)
            af = [afp.tile([128, NQ], BF16, tag=f"af{i}", name=f"af{i}") for i in range(8)]
            with (
                tc.tile_pool(name="work", bufs=4) as wkpool,
                tc.tile_pool(name="psS", bufs=3, space="PSUM") as psS,
                tc.tile_pool(name="psA", bufs=2, space="PSUM") as psA,
                tc.tile_pool(name="psR", bufs=2, space="PSUM") as psR,
            ):
                for h in range(HEADS):
                    off = (h % 2) * 64
                    kth = ktr[h // 2]
                    qtile = qt[h // 2]
                    for s in range(NSLOT):
                        trip = 4 * (s + 1)
                        navm = psA.tile([65, CH], F32, tag="navm", name="navm")
                        for jt in range(trip):
                            st = psS.tile([128, CH], F32, tag="st", name="st")
                            nc.tensor.matmul(
                                st[:, :],
                                kth[off:off + 64, jt * 128:(jt + 1) * 128],
                                qtile[off:off + 64, s * CH:(s + 1) * CH],
                                start=True,
                                stop=True,
                            )
                            if jt >= trip - 4:
                                # diagonal/padded region: exp f32 then masked cast
                                et = wkpool.tile([128, CH], F32, tag="et", name="et")
                                nc.scalar.activation(et[:, :], st[:, :], AF.Exp, scale=SCALE)
                                rhs = wkpool.tile([128, CH], BF16, tag="etm", name="etm")
                                col = s * 16 + jt
                                nc.vector.scalar_tensor_tensor(
                                    rhs[:, :],
                                    iota[:, :],
                                    ms[:, col:col + 1],
                                    et[:, :],
                                    ALU.is_ge,
                                    ALU.mult,
                                )
                            else:
                                # fully visible: exp straight to bf16
                                rhs = wkpool.tile([128, CH], BF16, tag="etb", name="etb")
                                nc.scalar.activation(rhs[:, :], st[:, :], AF.Exp, scale=SCALE)
                            nc.tensor.matmul(
                                navm[:, :],
                                vsb[jt][:, h * 65:(h + 1) * 65],
                                rhs[:, :],
                                start=(jt == 0),
                                stop=(jt == trip - 1),
                            )
                        rden = wkpool.tile([1, CH], F32, tag="rden", name="rden")
                        _act_reciprocal(nc, rden[:, :], navm[64:65, :])
                        rb = psR.tile([64, CH], F32, tag="rb", name="rb")
                        nc.tensor.matmul(rb[:, :], ones64[:, :], rden[:, :], start=True, stop=True)
                        nums = wkpool.tile([64, CH], F32, tag="nums", name="nums")
                        nc.scalar.activation(nums[:, :], navm[0:64, :], AF.Copy)
                        nc.vector.tensor_mul(
                            af[h // 2][off:off + 64, s * CH:(s + 1) * CH],
                            rb[:, :],
                            nums[:, :],
                        )

            # ---------------- P3: out-projection ----------------
            with (
                tc.tile_pool(name="wop", bufs=1) as wop,
                tc.tile_pool(name="wbp", bufs=1) as wbp,
                tc.tile_pool(name="ow", bufs=3) as owp,
                tc.tile_pool(name="psO", bufs=4, space="PSUM") as psO,
            ):
                wo = [wop.tile([128, DIM], BF16, tag=f"wo{i}", name=f"wo{i}") for i in range(8)]
                for i in range(8):
                    nc.sync.dma_start(wo[i][:, :], p_wbo[i * 128:(i + 1) * 128, :])
                wbias = wbp.tile([1, DIM], BF16, tag="wbias", name="wbias")
                nc.sync.dma_start(wbias[:, :], p_wbo[INNER:INNER + 1, :])
                for it in range(8):
                    for oc in range(2):
                        po = psO.tile([128, 512], F32, tag="po", name="po")
                        for ft in range(8):
                            nc.tensor.matmul(
                                po[:, :],
                                af[ft][:, it * 128:(it + 1) * 128],
                                wo[ft][:, oc * 512:(oc + 1) * 512],
                                start=(ft == 0),
                                stop=False,
                            )
                        nc.tensor.matmul(
                            po[:, :],
                            onesb[:, it * 128:(it + 1) * 128],
                            wbias[:, oc * 512:(oc + 1) * 512],
                            start=False,
                            stop=True,
                        )
                        ot = owp.tile([128, 512], F32, tag="ot", name="ot")
                        nc.scalar.activation(ot[:, :], po[:, :], AF.Copy)
                        nc.sync.dma_start(
                            p_out[it * 128:(it + 1) * 128, oc * 512:(oc + 1) * 512],
                            ot[:, :],
                        )
            afp.release()

    _split_multi_waits(nc)
    return nc


_GRAPH = None


def _get_graph():
    global _GRAPH
    if _GRAPH is None:
        _GRAPH = build_graph()
    return _GRAPH


def _core_row_blocks(c):
    # four 256-row position blocks per core
    par = c % 2
    return [2 * s + par for s in range(NSLOT)]


def kernel(x, mask, w_qkv, w_out, b_out):
    global LAST_RESULT
    x = np.asarray(x, dtype=np.float32)
    w_qkv = np.asarray(w_qkv, dtype=np.float32)
    w_out = np.asarray(w_out, dtype=np.float32)
    b_out = np.asarray(b_out, dtype=np.float32)

    nc = _get_graph()

    BF = ml_dtypes.bfloat16
    w_q = np.ascontiguousarray(w_qkv[:, :INNER].astype(BF))
    w_kv = np.ascontiguousarray(w_qkv[:, INNER:].astype(BF))
    wb = np.ascontiguousarray(np.vstack([w_out, b_out[None, :]]).astype(BF))
    iota = np.broadcast_to(np.arange(CH, dtype=np.float32), (128, CH)).copy()

    xT = [np.ascontiguousarray(x[b].T.astype(BF)) for b in range(B)]

    in_maps = []
    p = np.arange(128, dtype=np.float32)
    for c in range(8):
        b = c // 2
        blocks = _core_row_blocks(c)
        rows = np.concatenate([np.arange(pos * CH, (pos + 1) * CH) for pos in blocks])
        xTq = np.ascontiguousarray(x[b][rows].T.astype(BF))
        mstart = np.empty((128, NSLOT * 16), np.float32)
        for s in range(NSLOT):
            ibase = blocks[s] * CH
            for jt in range(16):
                mstart[:, s * 16 + jt] = jt * 128 + p - ibase
        in_maps.append(
            {
                "xT": xT[b],
                "xTq": xTq,
                "w_q": w_q,
                "w_kv": w_kv,
                "wb_out": wb,
                "mstart": mstart,
                "iota": iota,
            }
        )

    res = run_bass_kernel_spmd(nc, in_maps, list(range(8)))
    LAST_RESULT = res

    out = np.empty((B, N, DIM), dtype=np.float32)
    for c in range(8):
        b = c // 2
        r = res.results[c]["out"]
        for s, pos in enumerate(_core_row_blocks(c)):
            out[b, pos * CH:(pos + 1) * CH] = r[s * CH:(s + 1) * CH]
    return out


# revision 16
# speedup vs baseline: 1.2800x; 1.2800x over previous
"""Distributed causal multi-head attention for TRN2 (8 NeuronCores), v2.

Sharding: core c handles batch c//2. The batch's 2048 query rows are cut
into eight 256-row position blocks; even cores own blocks {0,2,4,6}, odd
cores {1,3,5,7}. Chunk slot s (0..3) statically runs 4(s+1) key tiles of
128, which covers both parities (even cores pad, odd cores fit exactly) —
identical instruction streams on all cores, causal skip of ~40% of the
rectangle. The causal mask is applied only on the last 4 key tiles of
each slot via (iota >= mstart) * exp with per-core mstart data.

Matmuls run in bf16 (fp32 PSUM accumulation); softmax stays max-free
(scores are O(1) for randn inputs) with the denominator accumulated by a
ones-column folded into the AV matmul.
"""

import sys

sys.path.insert(0, "/opt/trn_rl_repo")
import numpy as np
import ml_dtypes
import concourse.bass as bass
import concourse.mybir as mybir
import concourse.tile as tile
from concourse.vector_clock import ScopedClock
from concourse.bass_utils import run_bass_kernel_spmd
import concourse.bass_utils as _bu
import os as _os

if not getattr(_bu, "_ldw_patch", False):
    _bu._ldw_patch = True
    _orig_run_command = _bu.run_command

    def _run_command_ldw(cmd, *a, **kw):
        if _os.environ.get("BASS_LDW_OPT") and isinstance(cmd, list):
            cmd = [c.replace("--enable-ldw-opt=false", "--enable-ldw-opt=true") for c in cmd]
        return _orig_run_command(cmd, *a, **kw)

    _bu.run_command = _run_command_ldw

B, N, DIM = 4, 2048, 1024
HEADS, DH = 16, 64
INNER = HEADS * DH
SCALE = DH ** -0.5
NQ = 1024
CH = 256          # query chunk rows
NSLOT = 4         # chunks per core
F32 = mybir.dt.float32
BF16 = mybir.dt.bfloat16
AF = mybir.ActivationFunctionType
ALU = mybir.AluOpType

LAST_RESULT = None


def _drain_and_barrier_patched(self, tick_clock, wait_clock):
    nop_inst = self.nc.sync.nop(nofuse=True)
    wait_clock.add_sem_waits(nop_inst.ins, ScopedClock({None: tick_clock.global_clock}))
    si = nop_inst.ins.sync_info
    waits = list(si.on_wait or []) if si else []
    if len(waits) > 1:
        nop_inst.ins.sync_info = mybir.SyncInfo(
            on_wait=waits[:1], on_update=list(si.on_update or [])
        )
        for i in range(1, len(waits)):
            extra = self.nc.sync.nop(nofuse=True)
            extra.ins.sync_info = mybir.SyncInfo(on_wait=[waits[i]], on_update=[])
    self.nc.sync.drain()
    self.nc.all_engine_barrier()
    popped = self.nc._tile_sem_poison_stack.pop()
    assert popped is self._sem_poison
    self.nc.clear_and_free_semaphores(list(self.sems.allocated().values()))
    self.nc.all_engine_barrier()


tile.TileContext._drain_and_barrier = _drain_and_barrier_patched


def _split_multi_waits(nc):
    for f in nc.m.functions:
        for bb in f.blocks:
            insts = bb.instructions
            if not any(
                i.sync_info and i.sync_info.on_wait and len(i.sync_info.on_wait) > 1
                for i in insts
            ):
                continue
            new = []
            for inst in insts:
                si = inst.sync_info
                waits = list(si.on_wait) if si and si.on_wait else []
                if len(waits) > 1:
                    for w in waits[:-1]:
                        nop = mybir.InstNoOp(
                            name=nc.get_next_instruction_name(), ins=[], outs=[]
                        )
                        nop.engine = inst.engine
                        nop.sync_info = mybir.SyncInfo(on_wait=[w], on_update=[])
                        new.append(nop)
                    inst.sync_info = mybir.SyncInfo(
                        on_wait=[waits[-1]], on_update=list(si.on_update or [])
                    )
                new.append(inst)
            bb.instructions = new


def _act_reciprocal(nc, out, in_):
    # ACT LUT reciprocal (~1e-3 rel; fine for softmax denominators). bass's
    # activation() refuses Reciprocal, so emit the instruction directly.
    se = nc.scalar
    ins = [se.lower_ap(in_)]
    for v in (0.0, 1.0, 0.0):
        ins.append(mybir.ImmediateValue(dtype=mybir.dt.float32, value=v))
    return se.add_instruction(
        mybir.InstActivation(
            name=nc.get_next_instruction_name(),
            func=mybir.ActivationFunctionType.Reciprocal,
            ins=ins,
            outs=[se.lower_ap(out)],
        )
    )


def build_graph():
    nc = bass.Bass("TRN2", target_bir_lowering=False)

    p_xT = nc.declare_dram_parameter("xT", [DIM, N], BF16, isOutput=False)
    p_xTq = nc.declare_dram_parameter("xTq", [DIM, NQ], BF16, isOutput=False)
    p_wq = nc.declare_dram_parameter("w_q", [DIM, INNER], BF16, isOutput=False)
    p_wkv = nc.declare_dram_parameter("w_kv", [DIM, 2 * INNER], BF16, isOutput=False)
    p_wbo = nc.declare_dram_parameter("wb_out", [INNER + 1, DIM], BF16, isOutput=False)
    p_ms = nc.declare_dram_parameter("mstart", [128, NSLOT * 16], F32, isOutput=False)
    p_iota = nc.declare_dram_parameter("iota", [128, CH], F32, isOutput=False)
    p_out = nc.declare_dram_parameter("out", [NQ, DIM], F32, isOutput=True)

    with tile.TileContext(nc) as tc:
        with (
            tc.tile_pool(name="const", bufs=1) as cst,
            tc.tile_pool(name="qt", bufs=1) as qtp,
            tc.tile_pool(name="vsb", bufs=1) as vsp,
            tc.tile_pool(name="ktr", bufs=1) as ktrp,
        ):
            iota = cst.tile([128, CH], F32, tag="iota", name="iota")
            nc.sync.dma_start(iota[:, :], p_iota[:, :])
            ms = cst.tile([128, NSLOT * 16], F32, tag="ms", name="ms")
            nc.sync.dma_start(ms[:, :], p_ms[:, :])
            ones64 = cst.tile([1, 64], F32, tag="ones64", name="ones64")
            nc.vector.memset(ones64[:, :], 1.0)
            onesb = cst.tile([1, 1024], BF16, tag="onesb", name="onesb")
            nc.vector.memset(onesb[:, :], 1.0)

            qt = [qtp.tile([128, NQ], BF16, tag=f"qt{i}", name=f"qt{i}") for i in range(8)]
            vsb = [vsp.tile([128, HEADS * (DH + 1)], BF16, tag=f"v{i}", name=f"v{i}") for i in range(16)]
            ktr = [ktrp.tile([128, N], BF16, tag=f"kt{i}", name=f"kt{i}") for i in range(8)]

            # ---------------- P0a: QT = w_q.T @ xTq ----------------
            with (
                tc.tile_pool(name="xtq", bufs=1) as xtqp,
                tc.tile_pool(name="wqp", bufs=1) as wqp,
                tc.tile_pool(name="ps0", bufs=2, space="PSUM") as ps0,
            ):
                xtq = [xtqp.tile([128, NQ], BF16, tag=f"xtq{i}", name=f"xtq{i}") for i in range(8)]
                for i in range(8):
                    nc.sync.dma_start(xtq[i][:, :], p_xTq[i * 128:(i + 1) * 128, :])
                wq = [wqp.tile([128, INNER], BF16, tag=f"wq{i}", name=f"wq{i}") for i in range(8)]
                for i in range(8):
                    nc.sync.dma_start(wq[i][:, :], p_wq[i * 128:(i + 1) * 128, :])
                for ft in range(8):
                    for tc2 in range(2):
                        pq = ps0.tile([128, 512], F32, name="pq")
                        for kt in range(8):
                            nc.tensor.matmul(
                                pq[:, :],
                                wq[kt][:, ft * 128:(ft + 1) * 128],
                                xtq[kt][:, tc2 * 512:(tc2 + 1) * 512],
                                start=(kt == 0),
                                stop=(kt == 7),
                            )
                        nc.scalar.activation(
                            qt[ft][:, tc2 * 512:(tc2 + 1) * 512], pq[:, :], AF.Copy
                        )

            # ---------------- P0b/P0c: KT (resident) and V ----------------
            with tc.tile_pool(name="xt", bufs=1) as xtp:
                xt = [xtp.tile([128, N], BF16, tag=f"xt{i}", name=f"xt{i}") for i in range(8)]
                for i in range(8):
                    nc.sync.dma_start(xt[i][:, :], p_xT[i * 128:(i + 1) * 128, :])

                with (
                    tc.tile_pool(name="wkp", bufs=3) as wkp,
                    tc.tile_pool(name="ps1", bufs=1, space="PSUM") as ps1,
                ):
                    for ft in range(8):
                        pk = [ps1.tile([128, 512], F32, tag=f"pk{j}", name=f"pk{j}") for j in range(4)]
                        for kt in range(8):
                            wk = wkp.tile([128, 128], BF16, tag="wk", name="wk")
                            nc.sync.dma_start(
                                wk[:, :],
                                p_wkv[kt * 128:(kt + 1) * 128, ft * 128:(ft + 1) * 128],
                            )
                            for tc4 in range(4):
                                nc.tensor.matmul(
                                    pk[tc4][:, :],
                                    wk[:, :],
                                    xt[kt][:, tc4 * 512:(tc4 + 1) * 512],
                                    start=(kt == 0),
                                    stop=(kt == 7),
                                )
                        for tc4 in range(4):
                            nc.scalar.activation(
                                ktr[ft][:, tc4 * 512:(tc4 + 1) * 512], pk[tc4][:, :], AF.Copy
                            )

                with (
                    tc.tile_pool(name="wvp", bufs=3) as wvp,
                    tc.tile_pool(name="ps2", bufs=1, space="PSUM") as ps2,
                ):
                    for tgrp in range(2):
                        for fc in range(2):
                            pv = [ps2.tile([128, 512], F32, tag=f"pv{j}", name=f"pv{j}") for j in range(8)]
                            for kt in range(8):
                                wv = wvp.tile([128, 512], BF16, tag="wv", name="wv")
                                nc.sync.dma_start(
                                    wv[:, :],
                                    p_wkv[
                                        kt * 128:(kt + 1) * 128,
                                        INNER + fc * 512:INNER + (fc + 1) * 512,
                                    ],
                                )
                                for t8 in range(8):
                                    tt = tgrp * 8 + t8
                                    nc.tensor.matmul(
                                        pv[t8][:, :],
                                        xt[kt][:, tt * 128:(tt + 1) * 128],
                                        wv[:, :],
                                        start=(kt == 0),
                                        stop=(kt == 7),
                                    )
                            for t8 in range(8):
                                tt = tgrp * 8 + t8
                                dst = vsb[tt][
                                    :, fc * 8 * 65:(fc * 8 + 8) * 65
                                ].rearrange("p (g d) -> p g d", g=8)[:, :, 0:64]
                                src = pv[t8][:, :].rearrange("p (g d) -> p g d", g=8)
                                nc.vector.tensor_copy(dst, src)
                    for tt in range(16):
                        nc.vector.memset(
                            vsb[tt][:, :].rearrange("p (g d) -> p g d", g=16)[:, :, 64:65],
                            1.0,
                        )

            # ---------------- P1: attention ----------------
            afp = tc.alloc_tile_pool(name="af", bufs=1)
            af = [afp.tile([128, NQ], BF16, tag=f"af{i}", name=f"af{i}") for i in range(8)]
            with (
                tc.tile_pool(name="work", bufs=4) as wkpool,
                tc.tile_pool(name="psS", bufs=3, space="PSUM") as psS,
                tc.tile_pool(name="psA", bufs=2, space="PSUM") as psA,
                tc.tile_pool(name="psR", bufs=2, space="PSUM") as psR,
            ):
                for h in range(HEADS):
                    off = (h % 2) * 64
                    kth = ktr[h // 2]
                    qtile = qt[h // 2]
                    for s in range(NSLOT):
                        trip = 4 * (s + 1)
                        navm = psA.tile([65, CH], F32, tag="navm", name="navm")
                        for jt in range(trip):
                            st = psS.tile([128, CH], F32, tag="st", name="st")
                            nc.tensor.matmul(
                                st[:, :],
                                kth[off:off + 64, jt * 128:(jt + 1) * 128],
                                qtile[off:off + 64, s * CH:(s + 1) * CH],
                                start=True,
                                stop=True,
                            )
                            if jt >= trip - 4:
                                # diagonal/padded region: exp f32 then masked cast
                                et = wkpool.tile([128, CH], F32, tag="et", name="et")
                                nc.scalar.activation(et[:, :], st[:, :], AF.Exp, scale=SCALE)
                                rhs = wkpool.tile([128, CH], BF16, tag="etm", name="etm")
                                col = s * 16 + jt
                                nc.vector.scalar_tensor_tensor(
                                    rhs[:, :],
                                    iota[:, :],
                                    ms[:, col:col + 1],
                                    et[:, :],
                                    ALU.is_ge,
                                    ALU.mult,
                                )
                            else:
                                # fully visible: exp straight to bf16
                                rhs = wkpool.tile([128, CH], BF16, tag="etb", name="etb")
                                nc.scalar.activation(rhs[:, :], st[:, :], AF.Exp, scale=SCALE)
                            nc.tensor.matmul(
                                navm[:, :],
                                vsb[jt][:, h * 65:(h + 1) * 65],
                                rhs[:, :],
                                start=(jt == 0),
                                stop=(jt == trip - 1),
                            )
                        den = wkpool.tile([1, CH], F32, tag="den", name="den")
                        nc.scalar.activation(den[:, :], navm[64:65, :], AF.Copy)
                        rb = psR.tile([64, CH], F32, tag="rb", name="rb")
                        nc.tensor.matmul(rb[:, :], ones64[:, :], den[:, :], start=True, stop=True)
                        rbs = wkpool.tile([64, CH], F32, tag="rbs", name="rbs")
                        nc.vector.reciprocal(rbs[:, :], rb[:, :])
                        nums = wkpool.tile([64, CH], F32, tag="nums", name="nums")
                        nc.scalar.activation(nums[:, :], navm[0:64, :], AF.Copy)
                        nc.vector.tensor_mul(
                            af[h // 2][off:off + 64, s * CH:(s + 1) * CH],
                            rbs[:, :],
                            nums[:, :],
                        )

            # ---------------- P3: out-projection ----------------
            with (
                tc.tile_pool(name="wop", bufs=1) as wop,
                tc.tile_pool(name="wbp", bufs=1) as wbp,
                tc.tile_pool(name="ow", bufs=3) as owp,
                tc.tile_pool(name="psO", bufs=4, space="PSUM") as psO,
            ):
                wo = [wop.tile([128, DIM], BF16, tag=f"wo{i}", name=f"wo{i}") for i in range(8)]
                for i in range(8):
                    nc.sync.dma_start(wo[i][:, :], p_wbo[i * 128:(i + 1) * 128, :])
                wbias = wbp.tile([1, DIM], BF16, tag="wbias", name="wbias")
                nc.sync.dma_start(wbias[:, :], p_wbo[INNER:INNER + 1, :])
                for it in range(8):
                    for oc in range(2):
                        po = psO.tile([128, 512], F32, tag="po", name="po")
                        for ft in range(8):
                            nc.tensor.matmul(
                                po[:, :],
                                af[ft][:, it * 128:(it + 1) * 128],
                                wo[ft][:, oc * 512:(oc + 1) * 512],
                                start=(ft == 0),
                                stop=False,
                            )
                        nc.tensor.matmul(
                            po[:, :],
                            onesb[:, it * 128:(it + 1) * 128],
                            wbias[:, oc * 512:(oc + 1) * 512],
                            start=False,
                            stop=True,
                        )
                        ot = owp.tile([128, 512], F32, tag="ot", name="ot")
                        nc.scalar.activation(ot[:, :], po[:, :], AF.Copy)
                        nc.sync.dma_start(
                            p_out[it * 128:(it + 1) * 128, oc * 512:(oc + 1) * 512],
                            ot[:, :],
                        )
            afp.release()

    _split_multi_waits(nc)
    return nc


_GRAPH = None


def _get_graph():
    global _GRAPH
    if _GRAPH is None:
        _GRAPH = build_graph()
    return _GRAPH


def _core_row_blocks(c):
    # four 256-row position blocks per core
    par = c % 2
    return [2 * s + par for s in range(NSLOT)]


def kernel(x, mask, w_qkv, w_out, b_out):
    global LAST_RESULT
    x = np.asarray(x, dtype=np.float32)
    w_qkv = np.asarray(w_qkv, dtype=np.float32)
    w_out = np.asarray(w_out, dtype=np.float32)
    b_out = np.asarray(b_out, dtype=np.float32)

    nc = _get_graph()

    BF = ml_dtypes.bfloat16
    w_q = np.ascontiguousarray(w_qkv[:, :INNER].astype(BF))
    w_kv = np.ascontiguousarray(w_qkv[:, INNER:].astype(BF))
    wb = np.ascontiguousarray(np.vstack([w_out, b_out[None, :]]).astype(BF))
    iota = np.broadcast_to(np.arange(CH, dtype=np.float32), (128, CH)).copy()

    xT = [np.ascontiguousarray(x[b].T.astype(BF)) for b in range(B)]

    in_maps = []
    p = np.arange(128, dtype=np.float32)
    for c in range(8):
        b = c // 2
        blocks = _core_row_blocks(c)
        rows = np.concatenate([np.arange(pos * CH, (pos + 1) * CH) for pos in blocks])
        xTq = np.ascontiguousarray(x[b][rows].T.astype(BF))
        mstart = np.empty((128, NSLOT * 16), np.float32)
        for s in range(NSLOT):
            ibase = blocks[s] * CH
            for jt in range(16):
                mstart[:, s * 16 + jt] = jt * 128 + p - ibase
        in_maps.append(
            {
                "xT": xT[b],
                "xTq": xTq,
                "w_q": w_q,
                "w_kv": w_kv,
                "wb_out": wb,
                "mstart": mstart,
                "iota": iota,
            }
        )

    res = run_bass_kernel_spmd(nc, in_maps, list(range(8)))
    LAST_RESULT = res

    out = np.empty((B, N, DIM), dtype=np.float32)
    for c in range(8):
        b = c // 2
        r = res.results[c]["out"]
        for s, pos in enumerate(_core_row_blocks(c)):
            out[b, pos * CH:(pos + 1) * CH] = r[s * CH:(s + 1) * CH]
    return out
